# revision 16
# baseline (speedup 1.0000x reference)
"""Trainium2 Bass kernel for a locally-connected Conv2d (nn.Conv2dLocal).

Problem shapes (hardcoded):
  x      [B=64, Cin=32, H=32, W=32]  fp32
  weight [OH=30, OW=30, Cout=64, Cin=32, KH=3, KW=3] fp32 (per-location weights)
  bias   [Cout=64, OH=30, OW=30] fp32
  out    [B=64, Cout=64, OH=30, OW=30] fp32

Strategy: shard the 30 output rows across 8 cores (4 padded rows per core).
Per output row h, an SBUF tensor XH holds three input image rows laid out as
[(kh, ci) -> 97 partitions (96 + ones row for bias), (img col c, batch b) free].
For each image column c the stationary operand XH[:, c] is shared by up to
three (wl, kw) weight taps (wl + kw == c); the per-location weights stream as
the moving operand, packed on the host so each step's taps are contiguous.
Accumulation happens in PSUM: one bank holds 8 output locations (64 cols
each); per-element has_written bits make the first tap overwrite and later
taps accumulate.  The bias is folded in as a 97th contraction row whose
stationary value is 1.0.
"""

import os
import sys

import numpy as np

for _p in ("/opt/trn_rl_repo", "/root/.axon_site/_ro/trn_rl_repo"):
    if os.path.isdir(_p) and _p not in sys.path:
        sys.path.insert(0, _p)

import concourse.bass as bass  # noqa: E402
import concourse.tile as tile  # noqa: E402
from concourse import bacc, mybir  # noqa: E402
from concourse.bass_utils import run_bass_kernel_spmd  # noqa: E402

F32 = mybir.dt.float32

# problem constants
B, CI, H, W = 64, 32, 32, 32
CO = 64
KH = KW = 3
OH = OW = 30
NCORES = 8
RPC = 4  # padded output rows per core (8 * 4 = 32 >= 30)
OHP = NCORES * RPC  # 32
HPAD = OHP + KH - 1  # 34 padded input rows
K96 = KH * CI  # 96 contraction rows per kw tap
KP = K96 + 1  # + ones row for bias

# (c, j, wl) pair enumeration: j descending within each c so that psum slots
# (wl % 8) ascend within a segment, matching the moving-operand column order.
PAIRS = []
for _c in range(W):
    for _j in (2, 1, 0):
        _wl = _c - _j
        if 0 <= _wl < OW:
            PAIRS.append((_c, _j, _wl))
NPAIRS = len(PAIRS)  # 90
PAIR_IDX = {(c, j): i for i, (c, j, wl) in enumerate(PAIRS)}

# weight DMA chunks: pair-index ranges for c-blocks [0,8) [8,16) [16,24) [24,32)
CHUNKS = []
for cb in range(4):
    lo = min(i for i, (c, j, wl) in enumerate(PAIRS) if c // 8 == cb)
    hi = max(i for i, (c, j, wl) in enumerate(PAIRS) if c // 8 == cb) + 1
    CHUNKS.append((lo, hi))
# each (h, chunk) weight block is stored contiguously in DRAM (a strided DRAM
# read pins the whole transfer to a single SDMA engine; contiguous reads fan
# out across all 16)
CHUNK_OFF = []
_off = 0
for (p0, p1) in CHUNKS:
    CHUNK_OFF.append(_off)
    _off += KP * (p1 - p0) * CO
WROW = _off  # KP * NPAIRS * CO

# per-c matmul segments.  PSUM "pending zero" (has_written clear) is
# bank-granular on start=True, so a single matmul must touch slots that are
# uniformly first-write or uniformly accumulate:
#  - the j==0 tap (first write of slot wl==c) is always its own matmul,
#    start=True only when it is slot 0 (bank's first write -> safe to clear)
#  - j==2/1 taps accumulate (start=False), split at bank boundaries
SEGMENTS = {c: [] for c in range(W)}
for c in range(W):
    acc_pairs = [(i, PAIRS[i]) for i in range(NPAIRS)
                 if PAIRS[i][0] == c and PAIRS[i][1] > 0]
    seg = []
    for i, (cc, j, wl) in acc_pairs:
        if seg and (seg[-1][1][2] // 8) != (wl // 8):
            SEGMENTS[c].append(seg)
            seg = []
        seg.append((i, (cc, j, wl)))
    if seg:
        SEGMENTS[c].append(seg)
    if c < OW:  # first-write tap (c, j=0)
        SEGMENTS[c].append([(PAIR_IDX[(c, 0)], (c, 0, c))])

# drain bank `beta` right after processing column c == last write for the bank
DRAIN_AFTER_C = {}
for beta in range(4):
    last_wl = min(8 * beta + 7, OW - 1)
    DRAIN_AFTER_C.setdefault(last_wl + 2, []).append(beta)

_CACHED = {}


def _build_nc():
    """Build the single-core SPMD Bass program (identical on all 8 cores)."""
    from contextlib import ExitStack

    nc = bacc.Bacc("TRN2", target_bir_lowering=False, debug=False,
                   num_devices=NCORES)
    x_d = nc.dram_tensor("x", [(RPC + 2) * CI, W * B], F32,
                         kind="ExternalInput").ap()
    w_d = nc.dram_tensor("w", [RPC, WROW], F32,
                         kind="ExternalInput").ap()
    o_d = nc.dram_tensor("o", [B, RPC * OW * CO], F32,
                         kind="ExternalOutput").ap()

    with tile.TileContext(nc) as tc, ExitStack() as ctx:
        xpool = ctx.enter_context(tc.tile_pool(name="xh", bufs=2))
        wpool = ctx.enter_context(tc.tile_pool(name="wt", bufs=8))
        opool = ctx.enter_context(tc.tile_pool(name="ob", bufs=2))
        ppool = ctx.enter_context(
            tc.tile_pool(name="ps", bufs=8, space=bass.MemorySpace.PSUM))

        for h in range(RPC):
            out_sb = opool.tile([B, OW * CO], F32, name=f"ob_h{h}", tag="ob")
            # patch panel: partitions (kh,ci)=96 rows + ones row; free (c, b)
            xh = xpool.tile([KP, W * B], F32)
            nc.gpsimd.dma_start(xh[0:K96, :], x_d[CI * h:CI * h + K96, :])
            nc.gpsimd.memset(xh[K96:KP, :], 1.0)

            wtiles = []
            for ci_, (p0, p1) in enumerate(CHUNKS):
                wt = wpool.tile([KP, (p1 - p0) * CO], F32)
                blk = KP * (p1 - p0) * CO
                nc.scalar.dma_start(
                    wt[:], w_d[h, CHUNK_OFF[ci_]:CHUNK_OFF[ci_] + blk])
                wtiles.append((p0, wt))

            psums = {}
            for c in range(W):
                lhs = xh[:, c * B:(c + 1) * B]  # [97, 64] stationary
                for seg in SEGMENTS[c]:
                    i0 = seg[0][0]
                    npair = len(seg)
                    wl0 = seg[0][1][2]
                    beta = wl0 // 8
                    slot0 = wl0 % 8
                    j0seg = seg[0][1][1] == 0
                    start = j0seg and slot0 == 0
                    stop = (npair == 1 and seg[0][1][1] == 2
                            and (wl0 % 8 == 7 or wl0 == OW - 1))
                    cp0, wt = wtiles[PAIRS[i0][0] // 8]
                    rhs = wt[:, (i0 - cp0) * CO:(i0 - cp0 + npair) * CO]
                    if beta not in psums:
                        psums[beta] = ppool.tile([B, 512], F32,
                                                 name=f"ps_h{h}_b{beta}",
                                                 tag="ps")
                    out_ap = psums[beta][:, slot0 * CO:(slot0 + npair) * CO]
                    nc.tensor.matmul(out_ap, lhs, rhs, start=start, stop=stop)
                for beta in DRAIN_AFTER_C.get(c, []):
                    nslot = min(8, OW - 8 * beta)
                    src = psums.pop(beta)[:, :nslot * CO]
                    dst = out_sb[:, beta * 8 * CO:(beta * 8 + nslot) * CO]
                    if beta % 2 == 0:
                        nc.vector.tensor_copy(dst, src)
                    else:
                        nc.scalar.copy(dst, src)

            nc.gpsimd.dma_start(o_d[:, h * OW * CO:(h + 1) * OW * CO],
                                out_sb[:])
    nc.compile()
    return nc


def _prep_inputs(x, weight, bias):
    """Host-side shard + relayout. Returns in_maps for the 8 cores."""
    x = np.ascontiguousarray(np.asarray(x, dtype=np.float32))
    weight = np.ascontiguousarray(np.asarray(weight, dtype=np.float32))
    bias = np.ascontiguousarray(np.asarray(bias, dtype=np.float32))

    x_pad = np.zeros((B, CI, HPAD, W), np.float32)
    x_pad[:, :, :H, :] = x
    # [r, ci, w, b]
    x_t = np.ascontiguousarray(x_pad.transpose(2, 1, 3, 0))

    w_pad = np.zeros((OHP, OW, CO, CI, KH, KW), np.float32)
    w_pad[:OH] = weight
    # [oh, kh, ci, wl, kw, o] -> [oh, 96, wl, kw, o]
    w4 = w_pad.transpose(0, 4, 3, 1, 5, 2).reshape(OHP, K96, OW, KW, CO)
    bias_pad = np.zeros((CO, OHP, OW), np.float32)
    bias_pad[:, :OH] = bias
    bias_t = bias_pad.transpose(1, 2, 0)  # [oh, wl, o]

    wl_list = np.array([wl for (c, j, wl) in PAIRS])
    j_list = np.array([j for (c, j, wl) in PAIRS])
    w2 = np.zeros((OHP, KP, NPAIRS, CO), np.float32)
    w2[:, :K96, :, :] = w4[:, :, wl_list, j_list, :]
    j0 = j_list == 0
    w2[:, K96, j0, :] = bias_t[:, wl_list[j0], :]

    # per-(h, chunk) contiguous weight blocks
    w3 = np.empty((OHP, WROW), np.float32)
    for k, (p0, p1) in enumerate(CHUNKS):
        blk = KP * (p1 - p0) * CO
        w3[:, CHUNK_OFF[k]:CHUNK_OFF[k] + blk] = \
            w2[:, :, p0:p1, :].reshape(OHP, blk)

    in_maps = []
    for core in range(NCORES):
        r0 = RPC * core
        xc = np.ascontiguousarray(
            x_t[r0:r0 + RPC + 2].reshape((RPC + 2) * CI, W * B))
        wc = np.ascontiguousarray(w3[r0:r0 + RPC])
        in_maps.append({"x": xc, "w": wc})
    return in_maps


def _assemble(results):
    out = np.empty((B, CO, OH, OW), np.float32)
    for core in range(NCORES):
        oc = results[core]["o"].reshape(B, RPC, OW, CO).transpose(0, 3, 1, 2)
        r0 = RPC * core
        r1 = min(r0 + RPC, OH)
        if r1 > r0:
            out[:, :, r0:r1, :] = oc[:, :, :r1 - r0, :]
    return out


def run(x, weight, bias, trace=False, **trace_kwargs):
    """Build (cached), run on 8 cores, return (output, BassKernelResults)."""
    if "nc" not in _CACHED:
        _CACHED["nc"] = _build_nc()
    nc = _CACHED["nc"]
    in_maps = _prep_inputs(x, weight, bias)
    res = run_bass_kernel_spmd(nc, in_maps, list(range(NCORES)),
                               trace=trace, **trace_kwargs)
    return _assemble(res.results), res


def kernel(x, weight, bias):
    out, _ = run(x, weight, bias)
    return out


# revision 19
# speedup vs baseline: 4.2314x; 4.2314x over previous
"""Trainium2 Bass kernel for a locally-connected Conv2d (nn.Conv2dLocal).

Problem shapes (hardcoded):
  x      [B=64, Cin=32, H=32, W=32]  fp32
  weight [OH=30, OW=30, Cout=64, Cin=32, KH=3, KW=3] fp32 (per-location weights)
  bias   [Cout=64, OH=30, OW=30] fp32
  out    [B=64, Cout=64, OH=30, OW=30] fp32

Strategy: shard the 30 output rows across 8 cores (4 padded rows per core).
Per output row h, an SBUF tensor XH holds three input image rows laid out as
[(kh, ci) -> 97 partitions (96 + ones row for bias), (img col c, batch b) free].
For each image column c the stationary operand XH[:, c] is shared by up to
three (wl, kw) weight taps (wl + kw == c); the per-location weights stream as
the moving operand, packed on the host so each step's taps are contiguous.
Accumulation happens in PSUM: one bank holds 8 output locations (64 cols
each); per-element has_written bits make the first tap overwrite and later
taps accumulate.  The bias is folded in as a 97th contraction row whose
stationary value is 1.0.
"""

import os
import sys

import numpy as np

for _p in ("/opt/trn_rl_repo", "/root/.axon_site/_ro/trn_rl_repo"):
    if os.path.isdir(_p) and _p not in sys.path:
        sys.path.insert(0, _p)

import concourse.bass as bass  # noqa: E402
import concourse.tile as tile  # noqa: E402
from concourse import bacc, mybir  # noqa: E402
from concourse.bass_utils import run_bass_kernel_spmd  # noqa: E402

F32 = mybir.dt.float32

# problem constants
B, CI, H, W = 64, 32, 32, 32
CO = 64
KH = KW = 3
OH = OW = 30
NCORES = 8
RPC = 4  # padded output rows per core (8 * 4 = 32 >= 30)
OHP = NCORES * RPC  # 32
HPAD = OHP + KH - 1  # 34 padded input rows
K96 = KH * CI  # 96 contraction rows per kw tap
KP = K96 + 1  # + ones row for bias

# (c, j, wl) pair enumeration: j descending within each c so that psum slots
# (wl % 8) ascend within a segment, matching the moving-operand column order.
PAIRS = []
for _c in range(W):
    for _j in (2, 1, 0):
        _wl = _c - _j
        if 0 <= _wl < OW:
            PAIRS.append((_c, _j, _wl))
NPAIRS = len(PAIRS)  # 90
PAIR_IDX = {(c, j): i for i, (c, j, wl) in enumerate(PAIRS)}

# weight DMA chunks: pair-index ranges for c-blocks [0,8) [8,16) [16,24) [24,32)
CHUNKS = []
for cb in range(4):
    lo = min(i for i, (c, j, wl) in enumerate(PAIRS) if c // 8 == cb)
    hi = max(i for i, (c, j, wl) in enumerate(PAIRS) if c // 8 == cb) + 1
    CHUNKS.append((lo, hi))
# each (h, chunk) weight block is stored contiguously in DRAM, split into a
# 96-partition block + a 1-row bias block: HWDGE only fans a transfer across
# the 16 SDMA engines when the partition count is a multiple of 16; a 97-row
# transfer runs on a single engine at ~25 GB/s.
CHUNK_OFF = []   # offset of the 96-row main block of chunk k
_off = 0
for (p0, p1) in CHUNKS:
    CHUNK_OFF.append(_off)
    _off += K96 * (p1 - p0) * CO
BIAS_OFF = []    # offset of the 1-row bias block of chunk k
for (p0, p1) in CHUNKS:
    BIAS_OFF.append(_off)
    _off += (p1 - p0) * CO
WROW = _off  # KP * NPAIRS * CO

# per-c matmul segments.  PSUM "pending zero" (has_written clear) is
# bank-granular on start=True, so a single matmul must touch slots that are
# uniformly first-write or uniformly accumulate:
#  - the j==0 tap (first write of slot wl==c) is always its own matmul,
#    start=True only when it is slot 0 (bank's first write -> safe to clear)
#  - j==2/1 taps accumulate (start=False), split at bank boundaries
SEGMENTS = {c: [] for c in range(W)}
for c in range(W):
    acc_pairs = [(i, PAIRS[i]) for i in range(NPAIRS)
                 if PAIRS[i][0] == c and PAIRS[i][1] > 0]
    seg = []
    for i, (cc, j, wl) in acc_pairs:
        if seg and (seg[-1][1][2] // 8) != (wl // 8):
            SEGMENTS[c].append(seg)
            seg = []
        seg.append((i, (cc, j, wl)))
    if seg:
        SEGMENTS[c].append(seg)
    if c < OW:  # first-write tap (c, j=0)
        SEGMENTS[c].append([(PAIR_IDX[(c, 0)], (c, 0, c))])

# drain bank `beta` right after processing column c == last write for the bank
DRAIN_AFTER_C = {}
for beta in range(4):
    last_wl = min(8 * beta + 7, OW - 1)
    DRAIN_AFTER_C.setdefault(last_wl + 2, []).append(beta)

_CACHED = {}


def _build_nc():
    """Build the single-core SPMD Bass program (identical on all 8 cores)."""
    from contextlib import ExitStack

    nc = bacc.Bacc("TRN2", target_bir_lowering=False, debug=False,
                   num_devices=NCORES)
    x_d = nc.dram_tensor("x", [(RPC + 2) * CI, W * B], F32,
                         kind="ExternalInput").ap()
    w_d = nc.dram_tensor("w", [RPC, WROW], F32,
                         kind="ExternalInput").ap()
    o_d = nc.dram_tensor("o", [B, RPC * OW * CO], F32,
                         kind="ExternalOutput").ap()

    with tile.TileContext(nc) as tc, ExitStack() as ctx:
        xpool = ctx.enter_context(tc.tile_pool(name="xh", bufs=2))
        wpool = ctx.enter_context(tc.tile_pool(name="wt", bufs=8))
        opool = ctx.enter_context(tc.tile_pool(name="ob", bufs=2))
        ppool = ctx.enter_context(
            tc.tile_pool(name="ps", bufs=8, space=bass.MemorySpace.PSUM))

        for h in range(RPC):
            out_sb = opool.tile([B, OW * CO], F32, name=f"ob_h{h}", tag="ob")
            # patch panel: partitions (kh,ci)=96 rows + ones row; free (c, b)
            xh = xpool.tile([KP, W * B], F32)
            nc.gpsimd.dma_start(xh[0:K96, :], x_d[CI * h:CI * h + K96, :])
            nc.gpsimd.memset(xh[K96:KP, :], 1.0)

            wtiles = []
            for ci_, (p0, p1) in enumerate(CHUNKS):
                csz = (p1 - p0) * CO
                wt = wpool.tile([KP, csz], F32)
                nc.scalar.dma_start(
                    wt[0:K96, :],
                    w_d[h, CHUNK_OFF[ci_]:CHUNK_OFF[ci_] + K96 * csz])
                nc.scalar.dma_start(
                    wt[K96:KP, :],
                    w_d[h, BIAS_OFF[ci_]:BIAS_OFF[ci_] + csz])
                wtiles.append((p0, wt))

            psums = {}
            for c in range(W):
                lhs = xh[:, c * B:(c + 1) * B]  # [97, 64] stationary
                for seg in SEGMENTS[c]:
                    i0 = seg[0][0]
                    npair = len(seg)
                    wl0 = seg[0][1][2]
                    beta = wl0 // 8
                    slot0 = wl0 % 8
                    j0seg = seg[0][1][1] == 0
                    start = j0seg and slot0 == 0
                    stop = (npair == 1 and seg[0][1][1] == 2
                            and (wl0 % 8 == 7 or wl0 == OW - 1))
                    cp0, wt = wtiles[PAIRS[i0][0] // 8]
                    rhs = wt[:, (i0 - cp0) * CO:(i0 - cp0 + npair) * CO]
                    if beta not in psums:
                        psums[beta] = ppool.tile([B, 512], F32,
                                                 name=f"ps_h{h}_b{beta}",
                                                 tag="ps")
                    out_ap = psums[beta][:, slot0 * CO:(slot0 + npair) * CO]
                    nc.tensor.matmul(out_ap, lhs, rhs, start=start, stop=stop)
                for beta in DRAIN_AFTER_C.get(c, []):
                    nslot = min(8, OW - 8 * beta)
                    src = psums.pop(beta)[:, :nslot * CO]
                    dst = out_sb[:, beta * 8 * CO:(beta * 8 + nslot) * CO]
                    if beta % 2 == 0:
                        nc.vector.tensor_copy(dst, src)
                    else:
                        nc.scalar.copy(dst, src)

            nc.gpsimd.dma_start(o_d[:, h * OW * CO:(h + 1) * OW * CO],
                                out_sb[:])
    nc.compile()
    return nc


def _prep_inputs(x, weight, bias):
    """Host-side shard + relayout. Returns in_maps for the 8 cores."""
    x = np.ascontiguousarray(np.asarray(x, dtype=np.float32))
    weight = np.ascontiguousarray(np.asarray(weight, dtype=np.float32))
    bias = np.ascontiguousarray(np.asarray(bias, dtype=np.float32))

    x_pad = np.zeros((B, CI, HPAD, W), np.float32)
    x_pad[:, :, :H, :] = x
    # [r, ci, w, b]
    x_t = np.ascontiguousarray(x_pad.transpose(2, 1, 3, 0))

    w_pad = np.zeros((OHP, OW, CO, CI, KH, KW), np.float32)
    w_pad[:OH] = weight
    # [oh, kh, ci, wl, kw, o] -> [oh, 96, wl, kw, o]
    w4 = w_pad.transpose(0, 4, 3, 1, 5, 2).reshape(OHP, K96, OW, KW, CO)
    bias_pad = np.zeros((CO, OHP, OW), np.float32)
    bias_pad[:, :OH] = bias
    bias_t = bias_pad.transpose(1, 2, 0)  # [oh, wl, o]

    wl_list = np.array([wl for (c, j, wl) in PAIRS])
    j_list = np.array([j for (c, j, wl) in PAIRS])
    w2 = np.zeros((OHP, KP, NPAIRS, CO), np.float32)
    w2[:, :K96, :, :] = w4[:, :, wl_list, j_list, :]
    j0 = j_list == 0
    w2[:, K96, j0, :] = bias_t[:, wl_list[j0], :]

    # per-(h, chunk) contiguous weight blocks: 96-row main + 1-row bias
    w3 = np.empty((OHP, WROW), np.float32)
    for k, (p0, p1) in enumerate(CHUNKS):
        blk = K96 * (p1 - p0) * CO
        w3[:, CHUNK_OFF[k]:CHUNK_OFF[k] + blk] = \
            w2[:, :K96, p0:p1, :].reshape(OHP, blk)
        w3[:, BIAS_OFF[k]:BIAS_OFF[k] + (p1 - p0) * CO] = \
            w2[:, K96, p0:p1, :].reshape(OHP, (p1 - p0) * CO)

    in_maps = []
    for core in range(NCORES):
        r0 = RPC * core
        xc = np.ascontiguousarray(
            x_t[r0:r0 + RPC + 2].reshape((RPC + 2) * CI, W * B))
        wc = np.ascontiguousarray(w3[r0:r0 + RPC])
        in_maps.append({"x": xc, "w": wc})
    return in_maps


def _assemble(results):
    out = np.empty((B, CO, OH, OW), np.float32)
    for core in range(NCORES):
        oc = results[core]["o"].reshape(B, RPC, OW, CO).transpose(0, 3, 1, 2)
        r0 = RPC * core
        r1 = min(r0 + RPC, OH)
        if r1 > r0:
            out[:, :, r0:r1, :] = oc[:, :, :r1 - r0, :]
    return out


def run(x, weight, bias, trace=False, **trace_kwargs):
    """Build (cached), run on 8 cores, return (output, BassKernelResults)."""
    if "nc" not in _CACHED:
        _CACHED["nc"] = _build_nc()
    nc = _CACHED["nc"]
    in_maps = _prep_inputs(x, weight, bias)
    res = run_bass_kernel_spmd(nc, in_maps, list(range(NCORES)),
                               trace=trace, **trace_kwargs)
    return _assemble(res.results), res


def kernel(x, weight, bias):
    out, _ = run(x, weight, bias)
    return out
